# revision 55
# baseline (speedup 1.0000x reference)
"""Masked dot-product attention on 8 Trainium2 NeuronCores.

Problem: q,k,v [64, 1024, 64] f32, valid_lens [64] int32.
  scores = q @ k^T / 8, mask keys >= valid_len to -1e6, softmax, @ v.

Strategy (per core: 8 batches, pure data parallelism, no collectives):
  - Host prep: pre-transpose q,k to [D, S] (contraction dim on partitions),
    pre-zero v rows past valid_len and append the 0/1 mask as a 65th column
    (vm).  The masked softmax denominator then falls out of the same matmul
    that computes attn @ v.  valid_len==0 batches reproduce the reference's
    uniform-softmax by zeroing q (scores==0) and unmasking all keys.
  - Device, per key-tile j: scoresT[j,q] = kT_tile.T.T @ qT with fp16
    operands (keys on partitions, fp32 PSUM accumulate; fp16 streams 1 PE
    cycle/row vs fp32's 4 and bf16's 8-bit mantissa would cost ~2e-3
    output error), exp on ScalarE (scale=1/8, bias=-3 bounds the fp16
    range; it cancels between numerator and denominator), written fp16.
    Scores for two key tiles go out as adjacent matmuls on disjoint PE
    row groups (K=64) so they can execute concurrently.
  - attn@v runs with the exp'd tile as the stationary operand:
    po[128q, 65] += expT_chunk.T.T @ [v|mask]_tile per 128-query chunk,
    fp32 PSUM.  The fp16 weights ride the fast weight-load path and only 65
    columns stream per chunk, and the result lands queries-on-partitions:
    the softmax division is then a cheap [128, 4] reciprocal plus
    per-partition tensor_scalar multiplies -- no transposes, no broadcasts.
  - Chunk accumulation groups sharing a PSUM bank run sequentially (a
    group's start clears has_written for the whole bank), so the qc loop is
    outer and all exp tiles of a batch stay resident in SBUF.
  - DMA dispatch is the hidden serial resource (~0.6us per dma_start per
    sequencer): one vm load and one output store per batch, inputs on the
    Sync queue, outputs on the GpSimd queue.
  - Per-batch key tiles are truncated to ceil(valid/128): masked tail tiles
    contribute exactly zero, so they are skipped.  Batches are rank-sorted
    by valid_len and dealt one per core per slot (same baked schedule on
    every core), shortest slots first so epilogues hide under later compute.
"""

import numpy as np

import concourse.bass as bass
import concourse.bacc as bacc
import concourse.tile as tile
from concourse import mybir
from concourse import bass_utils

B, S, D = 64, 1024, 64
NCORES = 8
NB = B // NCORES  # batch slots per core
P = 128
NJT = S // P  # max key tiles per batch
W = D + 1  # v columns + mask column
F32 = mybir.dt.float32
F16 = mybir.dt.float16

TRACE = False  # set by test harness to capture an NTFF profile
LAST_RESULTS = None  # BassKernelResults stash for the harness

_program_cache = {}


def _av_steps(nc, po_pool, osb_pool, rec_pool, out, s, jt, exs, vm_t):
    """Yield one emission step at a time: 8 attn@v chunk-groups, then the
    normalization epilogue.  The caller interleaves these steps between the
    NEXT batch's score/exp pairs so the PE queue alternates between feeding
    ScalarE (scores) and draining it (attn@v).

    Output accumulators: 8 query-chunks of [128, 65] (cols 0..63 =
    unnormalized out rows, col 64 = denominator); a 65-wide chunk can't
    cross a PSUM bank so they're split 4+4 over two banks.  One pending
    accumulation group per PSUM bank at a time: a group's start clears
    has_written for the whole bank, so the 4 chunk groups sharing a bank
    run sequentially (alternating banks lets two overlap).
    """
    po = [po_pool.tile([P, 4 * W], F32, tag=f"po{h}", name=f"po{h}")
          for h in range(2)]
    order = [0, 4, 1, 5, 2, 6, 3, 7]  # alternate banks
    for qc in order:
        dst = po[qc // 4]
        col = (qc % 4) * W
        for j in range(jt):
            nc.tensor.matmul(
                dst[:, col:col + W],
                lhsT=exs[j][:, qc * P:(qc + 1) * P],
                rhs=vm_t[:, j * W:(j + 1) * W],
                start=(j == 0), stop=(j == jt - 1),
            )
            # fine-grained steps: never queue more than ~4 attn@v matmuls
            # ahead of the next batch's scores, or ScalarE loses exp slots
            if j % 4 == 3:
                yield
        yield
    osb = osb_pool.tile([P, 8 * D], F32, tag="osb", name="osb")
    for h in range(2):
        po3 = po[h].rearrange("p (c w) -> p c w", w=W)
        recp = rec_pool.tile([P, 4], F32, tag="rec", name="recp")
        nc.vector.reciprocal(out=recp, in_=po3[:, :, D])
        for i in range(4):
            qc = 4 * h + i
            nc.vector.tensor_scalar_mul(
                osb[:, qc * D:(qc + 1) * D],
                po3[:, i, 0:D],
                recp[:, i:i + 1],
            )
    eng = nc.gpsimd if s % 2 == 0 else nc.sync
    eng.dma_start(
        out=out[s].rearrange("(c p) d -> p c d", p=P),
        in_=osb.rearrange("p (c d) -> p c d", d=D),
    )
    yield


def _build_program(jt_counts):
    nc = bacc.Bacc("TRN2", target_bir_lowering=False, debug=False,
                   num_devices=NCORES)
    qT = nc.dram_tensor("qT", [NB, D, S], F16, kind="ExternalInput").ap()
    kT = nc.dram_tensor("kT", [NB, D, S], F16, kind="ExternalInput").ap()
    vm = nc.dram_tensor("vm", [NB, S, W], F16, kind="ExternalInput").ap()
    out = nc.dram_tensor("out", [NB, S, D], F32, kind="ExternalOutput").ap()

    with tile.TileContext(nc) as tc:
        with (
            tc.tile_pool(name="singles", bufs=1) as singles,
            tc.tile_pool(name="qk", bufs=3) as qk_pool,
            tc.tile_pool(name="vmp", bufs=4) as vm_pool,
            tc.tile_pool(name="ex", bufs=2 * NJT + 2) as ex_pool,
            tc.tile_pool(name="osb", bufs=2) as osb_pool,
            tc.tile_pool(name="rec", bufs=4) as rec_pool,
            tc.tile_pool(name="ps_s", bufs=3, space="PSUM") as ps_pool,
            tc.tile_pool(name="ps_o", bufs=1, space="PSUM") as po_pool,
        ):
            # exp(s/8 - 3): the -3 bounds the fp16 exp range; it cancels
            # between numerator and denominator.
            bias_t = singles.tile([P, 1], F32)
            nc.vector.memset(bias_t, -3.0)

            pending = None  # unfinished attn@v/epilogue of previous batch
            drip = 1
            for s in range(NB):
                jt = jt_counts[s]
                # q/k replicated into both partition halves (0-stride DMA
                # source) so score matmuls for two key-tiles can run
                # concurrently on PE row-groups (0..63) and (64..127).
                qT_t = qk_pool.tile([2 * D, S], F16, tag="qT")
                kT_t = qk_pool.tile([2 * D, S], F16, tag="kT")
                nc.sync.dma_start(out=qT_t[0:D, :], in_=qT[s])
                nc.gpsimd.dma_start(out=qT_t[D:2 * D, :], in_=qT[s])
                nc.sync.dma_start(out=kT_t[0:D, 0:jt * P],
                                  in_=kT[s, :, 0:jt * P])
                nc.gpsimd.dma_start(out=kT_t[D:2 * D, 0:jt * P],
                                    in_=kT[s, :, 0:jt * P])
                # All key tiles of vm in one DMA: [128, jt*65], tile j at
                # columns [j*65, (j+1)*65).
                vm_t = vm_pool.tile([P, NJT * W], F16, tag="vm", name="vm_t")
                nc.sync.dma_start(
                    out=vm_t.rearrange("p (j w) -> p j w", w=W)[:, 0:jt, :],
                    in_=vm[s, 0:jt * P, :].rearrange("(j p) w -> p j w", p=P),
                )
                # Score matmuls go out in row-group-interleaved pairs --
                # adjacent PE-queue entries on disjoint row groups execute
                # concurrently, so a pair of key tiles costs one tile's time.
                exs = []
                for m in range(0, jt, 2):
                    js = list(range(m, min(m + 2, jt)))
                    pss = [ps_pool.tile([P, S], F32, tag="ps", name="ps")
                           for _ in js]
                    for half in range(2):
                        for r, j in enumerate(js):
                            nc.tensor.matmul(
                                pss[r][:, half * 512:(half + 1) * 512],
                                lhsT=kT_t[r * D:(r + 1) * D,
                                          j * P:(j + 1) * P],
                                rhs=qT_t[r * D:(r + 1) * D,
                                         half * 512:(half + 1) * 512],
                                start=True, stop=True,
                                tile_position=(r * D, 0),
                            )
                    for r, j in enumerate(js):
                        ex = ex_pool.tile([P, S], F16, tag="ex", name="ex")
                        nc.scalar.activation(
                            out=ex, in_=pss[r],
                            func=mybir.ActivationFunctionType.Exp,
                            scale=0.125, bias=bias_t)
                        exs.append(ex)
                        # drain a sliver of the previous batch's attn@v
                        # after each exp (keeps ScalarE and PE both fed),
                        # paced to finish just before this batch's own attn@v
                        if pending is not None:
                            for _ in range(drip):
                                if next(pending, "done") == "done":
                                    pending = None
                                    break
                if pending is not None:
                    for _ in pending:
                        pass
                pending = _av_steps(nc, po_pool, osb_pool, rec_pool, out,
                                    s, jt, exs, vm_t)
                nsteps = 8 * ((jt + 3) // 4) + 1
                nxt = jt_counts[s + 1] if s + 1 < NB else jt
                drip = max(1, -(-nsteps // max(nxt, 1))) + 1
            for _ in pending:
                pass
    nc.compile()
    return nc


def kernel(q, k, v, valid_lens):
    global LAST_RESULTS
    q = np.array(q, dtype=np.float32, copy=True)
    k = np.asarray(k, dtype=np.float32)
    v = np.asarray(v, dtype=np.float32)
    vl = np.asarray(valid_lens).astype(np.int64)

    # valid_len == 0: reference's softmax over an all-masked row is uniform.
    # Zeroed q gives scores == 0 -> exp == 1 over all (unmasked) keys: same.
    valid_eff = np.where(vl <= 0, S, np.minimum(vl, S))
    q[vl <= 0] = 0.0

    mask = (np.arange(S)[None, :] < valid_eff[:, None]).astype(np.float32)
    qT = np.ascontiguousarray(q.transpose(0, 2, 1)).astype(np.float16)
    kT = np.ascontiguousarray(k.transpose(0, 2, 1)).astype(np.float16)
    vm = np.concatenate([v * mask[:, :, None], mask[:, :, None]], axis=2)
    vm = np.ascontiguousarray(vm).astype(np.float16)

    # Rank-sort batches by effective length; slot s takes one batch of rank
    # group [8s, 8s+8) per core, so the baked per-slot tile count wastes
    # little work.  Shortest slots run first (see module docstring).
    order = np.argsort(-valid_eff, kind="stable")
    assign = order.reshape(NB, NCORES)[::-1]  # ascending tile counts
    jt_counts = tuple(
        int(np.ceil(valid_eff[assign[s]].max() / P)) for s in range(NB)
    )

    nc = _program_cache.get(jt_counts)
    if nc is None:
        nc = _build_program(jt_counts)
        _program_cache[jt_counts] = nc

    in_maps = []
    for c in range(NCORES):
        bs = assign[:, c]
        in_maps.append({
            "qT": np.ascontiguousarray(qT[bs]),
            "kT": np.ascontiguousarray(kT[bs]),
            "vm": np.ascontiguousarray(vm[bs]),
        })
    res = bass_utils.run_bass_kernel_spmd(
        nc, in_maps, core_ids=list(range(NCORES)), trace=TRACE,
    )
    LAST_RESULTS = res

    out = np.empty((B, S, D), dtype=np.float32)
    for c in range(NCORES):
        o = res.results[c]["out"]
        for s in range(NB):
            out[assign[s, c]] = o[s]
    return out
